# revision 1
# baseline (speedup 1.0000x reference)
"""Trainium2 Bass kernel for the dense RandLA-Net block.

Reference computation (per batch b, point n, K=16 neighbors):
    enc   = [center(3), npos(3), rel(3), dist(1)]            # 10 dims
    rp    = relu(enc @ W_rel + b_rel)                        # 64
    f     = [rp, nfeat]                                      # 128
    att   = softmax_k(f @ W_att)                             # 128
    agg   = sum_k f * att                                    # 128
    out   = relu(agg @ W_glob + b_glob)                      # 128

Sharding: 8 cores = 4 batches x 2 point-halves (8192 points/core).
Within a core the 131072 (point, k) pairs are processed channel-major in
"block-k-major" column order: 16 blocks of 512 points, 16 k-slabs of 512
columns each.  Geometry (center/npos/dist) is computed with a grouped
GPSIMD ap_gather layout; neighbor features come from an SBUF-source
DMA transpose-gather; rp is one K=7 matmul per 512-column chunk against a
packed "enc" tile (rel is algebraically folded:  Wc*center + Wn*npos +
Wr*(npos-center) = (Wc-Wr)*center + (Wn+Wr)*npos).  The softmax-weighted
sums over k are PSUM-accumulated identity matmuls over the 16 k-slabs.
"""

import os
import sys

import numpy as np

sys.path.insert(0, "/opt/trn_rl_repo")

import ml_dtypes

import concourse.bass as bass
import concourse.tile as tile
from concourse import mybir, bacc
from concourse.bass_utils import run_bass_kernel_spmd

F32 = mybir.dt.float32
BF16 = mybir.dt.bfloat16
I16 = mybir.dt.int16
AF = mybir.ActivationFunctionType
OP = mybir.AluOpType
BF = ml_dtypes.bfloat16

B, C_IN, N, K = 4, 64, 16384, 16
D_REL, C_MID, C_OUT = 64, 128, 128
NP = N // 2            # points per core
PK = NP * K            # columns per core (131072)
NT = 16                # F tiles (= point blocks of 512)
LT = PK // NT          # 8192 cols per tile
NCH = 16               # chunks per tile
LC = 512               # chunk cols
ENC_F = PK // 4        # packed enc free size (32768)


def _build_kernel():
    nc = bacc.Bacc("TRN2", target_bir_lowering=False)

    # ---- DRAM tensors (per-core inputs) ----
    tabX = nc.dram_tensor("tabX", [128, N], F32, kind="ExternalInput")       # x duplicated (2-pack)
    tabG = nc.dram_tensor("tabG", [128, N], F32, kind="ExternalInput")       # grouped pos table
    posG = nc.dram_tensor("posG", [128, NP // 8], F32, kind="ExternalInput") # per-group own points
    gidxN = nc.dram_tensor("gidxN", [128, 1024], I16, kind="ExternalInput")  # geometry idx (wrapped)
    nfidx = nc.dram_tensor("nfidx", [128, 4096], I16, kind="ExternalInput")  # feature idx (2-pack)
    wattsw = nc.dram_tensor("wattsw", [128, 128], BF16, kind="ExternalInput")
    pswap = nc.dram_tensor("pswap", [128, 128], BF16, kind="ExternalInput")
    w7x4 = nc.dram_tensor("w7x4", [128, 64], BF16, kind="ExternalInput")
    watt = nc.dram_tensor("watt", [128, 128], BF16, kind="ExternalInput")
    wglob = nc.dram_tensor("wglob", [128, 128], BF16, kind="ExternalInput")
    ident = nc.dram_tensor("ident", [128, 128], BF16, kind="ExternalInput")
    sel3 = nc.dram_tensor("sel3", [128, 128], BF16, kind="ExternalInput")
    brel = nc.dram_tensor("brel", [128, 1], F32, kind="ExternalInput")
    bglob = nc.dram_tensor("bglob", [128, 1], F32, kind="ExternalInput")
    outp = nc.dram_tensor("outp", [128, NP], F32, kind="ExternalOutput")

    with tile.TileContext(nc) as tc:
        with tc.tile_pool(name="persist", bufs=1) as pp:
            enc = pp.tile([128, ENC_F], BF16)        # packed enc: subtile q at parts 32q..32q+6
            posG_sb = pp.tile([128, NP // 8], F32)
            w7_sb = pp.tile([128, 64], BF16)
            watt_sb = pp.tile([128, 128], BF16)
            wattsw_sb = pp.tile([128, 128], BF16)
            pswap_sb = pp.tile([128, 128], BF16)
            nc.sync.dma_start(out=wattsw_sb, in_=wattsw.ap())
            nc.sync.dma_start(out=pswap_sb, in_=pswap.ap())
            wglob_sb = pp.tile([128, 128], BF16)
            ident_sb = pp.tile([128, 128], BF16)
            sel3_sb = pp.tile([128, 128], BF16)
            brel_sb = pp.tile([128, 1], F32)
            bglob_sb = pp.tile([128, 1], F32)
            nc.sync.dma_start(out=posG_sb, in_=posG.ap())
            nc.sync.dma_start(out=w7_sb, in_=w7x4.ap())
            nc.sync.dma_start(out=watt_sb, in_=watt.ap())
            nc.sync.dma_start(out=wglob_sb, in_=wglob.ap())
            nc.sync.dma_start(out=ident_sb, in_=ident.ap())
            nc.sync.dma_start(out=sel3_sb, in_=sel3.ap())
            nc.sync.dma_start(out=brel_sb, in_=brel.ap())
            nc.sync.dma_start(out=bglob_sb, in_=bglob.ap())

            # ================= Phase B: geometry =================
            with tc.tile_pool(name="geo", bufs=1) as gp, \
                 tc.tile_pool(name="geops", bufs=2, space="PSUM") as gpsum:
                tabG_sb = gp.tile([128, N], F32)
                gidx_sb = gp.tile([128, 1024], I16)
                nc.sync.dma_start(out=tabG_sb, in_=tabG.ap())
                nc.sync.dma_start(out=gidx_sb, in_=gidxN.ap())
                for h in range(2):
                    gN = gp.tile([128, LT], F32, tag="gN")
                    nc.gpsimd.ap_gather(
                        out_ap=gN[:, :], in_ap=tabG_sb[:, :],
                        idxs_ap=gidx_sb[:, h * 512:(h + 1) * 512],
                        channels=128, num_elems=N, d=1, num_idxs=LT)
                    # s = npos - center   (center broadcast over k)
                    cen = posG_sb[:, h * 512:h * 512 + 512]
                    cen_b = bass.AP(tensor=cen.tensor, offset=cen.offset,
                                    ap=[[cen.ap[0][0], 128], [0, 16], [1, 512]])
                    s_t = gp.tile([128, LT], BF16, tag="s")
                    nc.vector.tensor_tensor(
                        out=s_t.rearrange("p (k i) -> p k i", i=512),
                        in0=gN.rearrange("p (k i) -> p k i", i=512),
                        in1=cen_b, op=OP.subtract)
                    m2 = s_t
                    nc.vector.tensor_mul(m2, s_t, s_t)
                    cenrep = gp.tile([128, 2048], F32, tag="cenrep")
                    crsrc = posG_sb[:, h * 512:h * 512 + 512]
                    crin = bass.AP(tensor=crsrc.tensor, offset=crsrc.offset,
                                   ap=[[crsrc.ap[0][0], 128], [0, 4], [1, 512]])
                    nc.vector.tensor_copy(
                        cenrep.rearrange("p (a i) -> p a i", i=512), crin)
                    dsb = gp.tile([128, LT], BF16, tag="dsb")
                    for cc in range(16):
                        psd = gpsum.tile([128, 512], F32, tag="psd")
                        nc.tensor.matmul(psd, sel3_sb, m2[:, cc * 512:(cc + 1) * 512],
                                         start=True, stop=True)
                        nc.scalar.activation(out=dsb[:, cc * 512:(cc + 1) * 512],
                                             in_=psd, func=AF.Sqrt)
                    # assembly DMAs into packed enc
                    for g in range(8):
                        ebase = g * 4096 + h * 2048
                        for r in range(4):
                            # center rows 32r..32r+2 (one contiguous cast DMA)
                            nc.gpsimd.dma_start(
                                out=enc[32 * r:32 * r + 3, ebase:ebase + 2048],
                                in_=cenrep[16 * g:16 * g + 3, :])
                            # npos rows 32r+3..32r+5
                            src_n = gN[16 * g:16 * g + 3, r * 512:r * 512 + 512]
                            src_n = bass.AP(tensor=src_n.tensor, offset=src_n.offset,
                                            ap=[[src_n.ap[0][0], 3], [2048, 4], [1, 512]])
                            dst_n = enc[32 * r + 3:32 * r + 6, ebase:ebase + 2048]
                            dst_n = bass.AP(tensor=dst_n.tensor, offset=dst_n.offset,
                                            ap=[[dst_n.ap[0][0], 3], [512, 4], [1, 512]])
                            nc.gpsimd.dma_start(out=dst_n, in_=src_n)
                            # dist row 32r+6
                            src_d = dsb[16 * g + 6:16 * g + 7, r * 512:r * 512 + 512]
                            src_d = bass.AP(tensor=src_d.tensor, offset=src_d.offset,
                                            ap=[[src_d.ap[0][0], 1], [2048, 4], [1, 512]])
                            dst_d = enc[32 * r + 6:32 * r + 7, ebase:ebase + 2048]
                            dst_d = bass.AP(tensor=dst_d.tensor, offset=dst_d.offset,
                                            ap=[[dst_d.ap[0][0], 1], [512, 4], [1, 512]])
                            nc.sync.dma_start(out=dst_d, in_=src_d)

            # ================= Phase C: main loop =================
            with tc.tile_pool(name="main", bufs=1) as mp, \
                 tc.tile_pool(name="ftiles", bufs=2) as fp, \
                 tc.tile_pool(name="chunks", bufs=2) as cp, \
                 tc.tile_pool(name="mps", bufs=2, space="PSUM") as mpsum, \
                 tc.tile_pool(name="accps", bufs=1, space="PSUM") as apsum:
                tabX_sb = mp.tile([128, N], F32)
                nfidx_sb = mp.tile([128, 4096], I16)
                nc.sync.dma_start(out=tabX_sb, in_=tabX.ap())
                nc.sync.dma_start(out=nfidx_sb, in_=nfidx.ap())

                for t in range(NT):
                    ft = fp.tile([128, LT], BF16, tag="ft")
                    for hf in range(2):
                        gX = fp.tile([128, 2048], F32, tag="gX")
                        nc.gpsimd.ap_gather(
                            out_ap=gX[:, :], in_ap=tabX_sb[:, :],
                            idxs_ap=nfidx_sb[:, t * 256 + hf * 128:t * 256 + (hf + 1) * 128],
                            channels=128, num_elems=N, d=1, num_idxs=2048)
                        # rows 0-63: swapped half (ft cols 0:4096); rows 64-127: canonical
                        nc.gpsimd.dma_start(
                            out=ft[0:64, hf * 2048:(hf + 1) * 2048], in_=gX[0:64, :])
                        nc.gpsimd.dma_start(
                            out=ft[64:128, 4096 + hf * 2048:4096 + (hf + 1) * 2048],
                            in_=gX[64:128, :])
                    if True:
                        ps_den = apsum.tile([128, 512], F32, tag="den")
                        ps_num = apsum.tile([128, 512], F32, tag="num")
                        for pr in range(NCH // 2):
                            swapped = pr < 4
                            rbase = 64 if swapped else 0
                            widt = wattsw_sb if swapped else watt_sb
                            pacc = pswap_sb if swapped else ident_sb
                            ps_s = mpsum.tile([128, 1024], F32, tag="sc")
                            pcols = slice(pr * 1024, (pr + 1) * 1024)
                            for ci in range(2):
                                cc = 2 * pr + ci
                                q = cc % 4
                                eoff = (t * 4 + cc // 4) * 512
                                cols = slice(cc * 512, (cc + 1) * 512)
                                ps_rp = mpsum.tile([128, 512], F32, tag="rp")
                                nc.tensor.matmul(ps_rp[rbase:rbase + 64, :],
                                                 w7_sb[32 * q:32 * q + 7, :],
                                                 enc[32 * q:32 * q + 7, eoff:eoff + 512],
                                                 start=True, stop=True,
                                                 tile_position=(32 * q, rbase))
                                if cc % 2 == 0:
                                    nc.scalar.activation(out=ft[rbase:rbase + 64, cols],
                                                         in_=ps_rp[rbase:rbase + 64, :],
                                                         func=AF.Relu,
                                                         bias=brel_sb[rbase:rbase + 64, :],
                                                         scale=1.0)
                                else:
                                    nc.vector.tensor_scalar(out=ft[rbase:rbase + 64, cols],
                                                            in0=ps_rp[rbase:rbase + 64, :],
                                                            scalar1=brel_sb[rbase:rbase + 64, :],
                                                            scalar2=0.0,
                                                            op0=OP.add, op1=OP.max)
                                nc.tensor.matmul(ps_s[:, ci * 512:(ci + 1) * 512],
                                                 widt, ft[:, cols],
                                                 start=True, stop=True)
                            eu = cp.tile([128, 2048], BF16, tag="eu")
                            nc.scalar.activation(out=eu[:, 0:1024], in_=ps_s, func=AF.Exp)
                            nc.vector.tensor_mul(eu[:, 1024:2048], ft[:, pcols],
                                                 eu[:, 0:1024])
                            for ci in range(2):
                                cc = 2 * pr + ci
                                nc.tensor.matmul(ps_den, pacc,
                                                 eu[:, ci * 512:(ci + 1) * 512],
                                                 start=(cc == 0), stop=(cc == NCH - 1),
                                                 skip_group_check=True)
                                nc.tensor.matmul(ps_num, pacc,
                                                 eu[:, 1024 + ci * 512:1024 + (ci + 1) * 512],
                                                 start=(cc == 0), stop=(cc == NCH - 1),
                                                 skip_group_check=True)
                        rcp = cp.tile([128, 512], F32, tag="rcp")
                        nc.vector.reciprocal(rcp, ps_den)
                        agg = cp.tile([128, 512], BF16, tag="agg")
                        nc.vector.tensor_mul(agg, ps_num, rcp)
                        ps_o = mpsum.tile([128, 512], F32, tag="rp")
                        nc.tensor.matmul(ps_o, wglob_sb, agg, start=True, stop=True)
                        osb = cp.tile([128, 512], F32, tag="osb")
                        nc.scalar.activation(out=osb, in_=ps_o, func=AF.Relu,
                                             bias=bglob_sb, scale=1.0)
                        nc.sync.dma_start(out=outp.ap()[:, t * 512:(t + 1) * 512], in_=osb)
    nc.compile()
    return nc


_NC = None


def _get_nc():
    global _NC
    if _NC is None:
        _NC = _build_kernel()
    return _NC


def _prep_core(core, x, pos, neigh, Wc, Wn, Wr, wd, W_att, W_glob, b_rel, b_glob):
    b = core // 2
    half = core % 2
    P0 = half * NP
    nb = neigh[b][P0:P0 + NP].astype(np.int64)      # [NP, K]
    xb = x[b]                                        # [64, N] f32
    posb = pos[b]                                    # [N, 3] f32

    # feature table: x duplicated on both partition halves
    tabX = np.concatenate([xb, xb], axis=0).astype(np.float32)   # [128, N]

    # tabG: rows 16g+j (j<3) = pos component j
    tabG = np.zeros((128, N), np.float32)
    for j in range(3):
        tabG[j::16, :] = posb[:, j][None, :]
    # posG: [16g+j, c] = pos comp j of point P0 + g*1024 + c
    posG = np.zeros((128, NP // 8), np.float32)
    pl = posb[P0:P0 + NP]
    for g in range(8):
        for j in range(3):
            posG[16 * g + j] = pl[g * 1024:(g + 1) * 1024, j]

    # geometry idx: block t2 = 2g+h; j in [0, 8192): k = j//512, i = j%512
    A = nb.reshape(16, 512, 16)                      # [block, i, k]
    V = A.transpose(0, 2, 1).reshape(16, LT)         # [block, j] j = k*512+i
    V2 = V.reshape(16, 512, 16).transpose(0, 2, 1)   # [block, j%16, j//16]
    gidxN = np.zeros((128, 1024), np.int16)
    for g in range(8):
        gidxN[16 * g:16 * g + 16, 0:512] = V2[2 * g]
        gidxN[16 * g:16 * g + 16, 512:1024] = V2[2 * g + 1]

    # nfeat idx (4-pack): inst t2, group g gathers chunk m=g//2 of its 16384-col range
    cs = np.arange(PK)
    t_ = cs >> 13
    k_ = (cs >> 9) & 15
    i_ = cs & 511
    s_nf = nb[t_ * 512 + i_, k_]
    nfidx = np.zeros((128, 4096), np.int16)
    for t in range(16):
        for hf in range(2):
            for g in range(8):
                m = g // 4
                base = t * 8192 + m * 4096 + hf * 2048
                seg = s_nf[base:base + 2048]
                nfidx[16 * g:16 * g + 16,
                      t * 256 + hf * 128:t * 256 + (hf + 1) * 128] = \
                    seg.reshape(128, 16).T.astype(np.int16)

    perm = (np.arange(128) + 64) % 128
    w7 = np.concatenate([Wc - Wr, Wn + Wr, wd], axis=0)  # [7, 64]
    w7x4 = np.zeros((128, 64), dtype=BF)
    for q in range(4):
        w7x4[32 * q:32 * q + 7] = w7.astype(BF)
    sel3 = np.zeros((128, 128), dtype=BF)
    for g in range(8):
        for j in range(3):
            sel3[16 * g + j, 16 * g + 6] = 1
    ident = np.eye(128, dtype=BF)

    return {
        "tabX": tabX, "tabG": tabG, "posG": posG,
        "gidxN": gidxN, "nfidx": nfidx,
        "w7x4": w7x4, "watt": W_att.astype(BF), "wglob": W_glob.astype(BF),
        "ident": ident, "sel3": sel3,
        "wattsw": W_att[np.ix_(perm, perm)].astype(BF),
        "pswap": np.roll(np.eye(128, dtype=np.float32), 64, axis=0).astype(BF),
        "brel": np.concatenate([b_rel, b_rel]).reshape(128, 1).astype(np.float32),
        "bglob": b_glob.reshape(128, 1).astype(np.float32),
    }


def kernel(x, pos, neigh_idx, W_rel, b_rel, W_att, W_glob, b_glob, **kw):
    x = np.ascontiguousarray(np.asarray(x, dtype=np.float32))
    pos = np.ascontiguousarray(np.asarray(pos, dtype=np.float32))
    neigh = np.asarray(neigh_idx)
    W_rel = np.asarray(W_rel, dtype=np.float32)
    W_att = np.asarray(W_att, dtype=np.float32)
    W_glob = np.asarray(W_glob, dtype=np.float32)
    b_rel = np.asarray(b_rel, dtype=np.float32)
    b_glob = np.asarray(b_glob, dtype=np.float32)
    Wc, Wn, Wr, wd = W_rel[0:3], W_rel[3:6], W_rel[6:9], W_rel[9:10]

    nc = _get_nc()
    in_maps = [
        _prep_core(core, x, pos, neigh, Wc, Wn, Wr, wd, W_att, W_glob, b_rel, b_glob)
        for core in range(8)
    ]
    res = run_bass_kernel_spmd(nc, in_maps, core_ids=list(range(8)))
    out = np.zeros((B, C_OUT, N), np.float32)
    for core in range(8):
        b = core // 2
        P0 = (core % 2) * NP
        out[b, :, P0:P0 + NP] = res.results[core]["outp"]
    return out



# revision 3
# speedup vs baseline: 1.6151x; 1.6151x over previous
"""Trainium2 Bass kernel for the dense RandLA-Net block.

Reference computation (per batch b, point n, K=16 neighbors):
    enc   = [center(3), npos(3), rel(3), dist(1)]            # 10 dims
    rp    = relu(enc @ W_rel + b_rel)                        # 64
    f     = [rp, nfeat]                                      # 128
    att   = softmax_k(f @ W_att)                             # 128
    agg   = sum_k f * att                                    # 128
    out   = relu(agg @ W_glob + b_glob)                      # 128

Sharding: 8 cores = 4 batches x 2 point-halves (8192 points/core).
Per core the 131072 (point, k) pairs are processed in 16 tiles of 512
points (8192 k-major columns each).  One SWDGE transposed dma_gather per
tile fetches, for each column, a 256-byte row of a packed DRAM table
(bf16: x features at rows 0:64, pos at rows 64:67) straight into the
f-layout [128, 8192] tile.  dist is computed in a k-on-partitions
[48, 512] layout (free-size 512), summed by one matmul, sqrt'd and
DMA'd back into row 67; rp is one 7-row matmul per 512-col chunk
(rel folded: Wc*center + Wn*npos + Wr*(npos-center) = (Wc-Wr)*center +
(Wn+Wr)*npos) whose relu overwrites rows 64:128.  The channel order is
f = [nfeat; rp] everywhere (W_att row+col permuted, W_glob row permuted
host-side).  Softmax-weighted sums over k are PSUM-accumulated identity
matmuls.
"""

import sys

import numpy as np

sys.path.insert(0, "/opt/trn_rl_repo")

import ml_dtypes

import concourse.bass as bass
import concourse.tile as tile
from concourse import mybir, bacc
from concourse.bass_utils import run_bass_kernel_spmd

F32 = mybir.dt.float32
BF16 = mybir.dt.bfloat16
I16 = mybir.dt.int16
AF = mybir.ActivationFunctionType
OP = mybir.AluOpType
BF = ml_dtypes.bfloat16

B, C_IN, N, K = 4, 64, 16384, 16
D_REL, C_MID, C_OUT = 64, 128, 128
NP = N // 2            # points per core
NT = 16                # tiles (point blocks of 512)
TP = NP // NT          # 512 points per tile
PKT = TP * K           # 8192 columns per tile
NCH = 16               # 512-col chunks per tile


def _ap3(t2d, n_idx):
    # [128, n] 2D AP -> [128, 1, n] 3D AP for dma_gather transpose out
    return bass.AP(tensor=t2d.tensor, offset=t2d.offset,
                   ap=[[t2d.ap[0][0], 128], [n_idx, 1], [1, n_idx]])


def _view(t, apl):
    return bass.AP(tensor=t.tensor, offset=t.offset, ap=apl)


def _build_kernel():
    nc = bacc.Bacc("TRN2", target_bir_lowering=False)

    tabT = nc.dram_tensor("tabT", [N, 128], BF16, kind="ExternalInput")
    idxg = nc.dram_tensor("idxg", [128, NP], I16, kind="ExternalInput")
    posC = nc.dram_tensor("posC", [3, NP], BF16, kind="ExternalInput")
    w7 = nc.dram_tensor("w7", [128, 64], BF16, kind="ExternalInput")
    watt = nc.dram_tensor("watt", [128, 128], BF16, kind="ExternalInput")
    wglob = nc.dram_tensor("wglob", [128, 128], BF16, kind="ExternalInput")
    w48 = nc.dram_tensor("w48", [128, 16], BF16, kind="ExternalInput")
    ident = nc.dram_tensor("ident", [128, 128], BF16, kind="ExternalInput")
    brel = nc.dram_tensor("brel", [128, 1], F32, kind="ExternalInput")
    bglob = nc.dram_tensor("bglob", [128, 1], F32, kind="ExternalInput")
    outp = nc.dram_tensor("outp", [128, NP], F32, kind="ExternalOutput")

    with tile.TileContext(nc) as tc:
        with tc.tile_pool(name="persist", bufs=1) as pp:
            idx_sb = pp.tile([128, NP], I16)
            posC_sb = pp.tile([3, NP], BF16)
            cen48 = pp.tile([48, NP], BF16)
            w7_sb = pp.tile([128, 64], BF16)
            watt_sb = pp.tile([128, 128], BF16)
            wglob_sb = pp.tile([128, 128], BF16)
            w48_sb = pp.tile([128, 16], BF16)
            ident_sb = pp.tile([128, 128], BF16)
            brel_sb = pp.tile([128, 1], F32)
            bglob_sb = pp.tile([128, 1], F32)
            nc.sync.dma_start(out=idx_sb, in_=idxg.ap())
            nc.sync.dma_start(out=posC_sb, in_=posC.ap())
            nc.sync.dma_start(out=w7_sb, in_=w7.ap())
            nc.sync.dma_start(out=watt_sb, in_=watt.ap())
            nc.sync.dma_start(out=wglob_sb, in_=wglob.ap())
            nc.sync.dma_start(out=w48_sb, in_=w48.ap())
            nc.sync.dma_start(out=ident_sb, in_=ident.ap())
            nc.sync.dma_start(out=brel_sb, in_=brel.ap())
            nc.sync.dma_start(out=bglob_sb, in_=bglob.ap())
            # cen48[16j+k, p] = posC[j, p]  (center replicated over k)
            src = posC_sb[0:3, :]
            nc.sync.dma_start(
                out=cen48[0:48, :],
                in_=_view(src, [[src.ap[0][0], 3], [0, 16], [1, NP]]))

            with tc.tile_pool(name="gpool", bufs=2) as gp, \
                 tc.tile_pool(name="epool", bufs=2) as ep, \
                 tc.tile_pool(name="spool", bufs=2) as sp, \
                 tc.tile_pool(name="mps", bufs=2, space="PSUM") as mpsum, \
                 tc.tile_pool(name="accps", bufs=1, space="PSUM") as apsum:

                def gather(t):
                    g = gp.tile([128, PKT], BF16, tag="G")
                    nc.gpsimd.dma_gather(
                        out_ap=_ap3(g[:, :], PKT), in_ap=tabT.ap(),
                        idxs_ap=idx_sb[:, t * TP:(t + 1) * TP],
                        num_idxs=PKT, num_idxs_reg=PKT, elem_size=128,
                        transpose=True, single_packet=False)
                    return g

                g_cur = gather(0)
                for t in range(NT):
                    g_nxt = gather(t + 1) if t + 1 < NT else None
                    g = g_cur

                    # ---- geometry: dist into G row 67, center into 68:71
                    cb_src = posC_sb[0:3, t * TP:(t + 1) * TP]
                    cb_dst = g[68:71, :]
                    nc.sync.dma_start(
                        out=_view(cb_dst, [[cb_dst.ap[0][0], 3], [TP, 16],
                                           [1, TP]]),
                        in_=_view(cb_src, [[cb_src.ap[0][0], 3], [0, 16],
                                           [1, TP]]))
                    n48 = sp.tile([48, TP], BF16, tag="n48")
                    np_src = g[64:67, :]
                    nc.sync.dma_start(
                        out=n48[0:48, :],
                        in_=_view(np_src, [[np_src.ap[0][0], 3], [TP, 16],
                                           [1, TP]]))
                    nc.vector.tensor_tensor(
                        out=n48, in0=n48,
                        in1=cen48[0:48, t * TP:(t + 1) * TP], op=OP.subtract)
                    nc.vector.tensor_mul(n48, n48, n48)
                    psd = mpsum.tile([128, TP], F32, tag="rp")
                    nc.tensor.matmul(psd[0:16, :], w48_sb[0:48, :],
                                     n48[0:48, :], start=True, stop=True)
                    dsb = sp.tile([16, TP], BF16, tag="dsb")
                    nc.scalar.activation(out=dsb[0:16, :], in_=psd[0:16, :],
                                         func=AF.Sqrt)
                    d_dst = g[67:68, :]
                    nc.sync.dma_start(
                        out=_view(d_dst, [[d_dst.ap[0][0], 1], [TP, 16],
                                          [1, TP]]),
                        in_=dsb[0:16, :])

                    # ---- phase A: rp chunks (PE keeps w7 resident)
                    for cc in range(NCH):
                        cols = slice(cc * TP, (cc + 1) * TP)
                        ps_rp = mpsum.tile([128, TP], F32, tag="rp")
                        nc.tensor.matmul(ps_rp[64:128, :], w7_sb[64:71, :],
                                         g[64:71, cols], start=True, stop=True,
                                         tile_position=(64, 64))
                        if cc % 2 == 0:
                            nc.scalar.activation(out=g[64:128, cols],
                                                 in_=ps_rp[64:128, :],
                                                 func=AF.Relu,
                                                 bias=brel_sb[64:128, :],
                                                 scale=1.0)
                        else:
                            nc.vector.tensor_scalar(out=g[64:128, cols],
                                                    in0=ps_rp[64:128, :],
                                                    scalar1=brel_sb[64:128, :],
                                                    scalar2=0.0,
                                                    op0=OP.add, op1=OP.max)

                    # ---- phase B: attention scores / exp / f*e
                    eu = ep.tile([128, 2 * PKT], BF16, tag="eu")
                    for pr in range(NCH // 2):
                        pcols = slice(pr * 1024, (pr + 1) * 1024)
                        ps_s = mpsum.tile([128, 1024], F32, tag="sc")
                        for ci in range(2):
                            cc = 2 * pr + ci
                            cols = slice(cc * TP, (cc + 1) * TP)
                            nc.tensor.matmul(ps_s[:, ci * TP:(ci + 1) * TP],
                                             watt_sb, g[:, cols],
                                             start=True, stop=True)
                        nc.scalar.activation(out=eu[:, pcols], in_=ps_s,
                                             func=AF.Exp)
                        nc.vector.tensor_mul(
                            eu[:, PKT + pr * 1024:PKT + (pr + 1) * 1024],
                            g[:, pcols], eu[:, pcols])

                    # ---- phase C: accumulate num/den over k
                    ps_den = apsum.tile([128, TP], F32, tag="den")
                    ps_num = apsum.tile([128, TP], F32, tag="num")
                    for cc in range(NCH):
                        nc.tensor.matmul(ps_den, ident_sb,
                                         eu[:, cc * TP:(cc + 1) * TP],
                                         start=(cc == 0), stop=(cc == NCH - 1),
                                         skip_group_check=True)
                        nc.tensor.matmul(ps_num, ident_sb,
                                         eu[:, PKT + cc * TP:PKT + (cc + 1) * TP],
                                         start=(cc == 0), stop=(cc == NCH - 1),
                                         skip_group_check=True)

                    rcp = sp.tile([128, TP], F32, tag="rcp")
                    nc.vector.reciprocal(rcp, ps_den)
                    agg = sp.tile([128, TP], BF16, tag="agg")
                    nc.vector.tensor_mul(agg, ps_num, rcp)
                    ps_o = mpsum.tile([128, TP], F32, tag="rp")
                    nc.tensor.matmul(ps_o, wglob_sb, agg, start=True, stop=True)
                    osb = sp.tile([128, TP], F32, tag="osb")
                    nc.scalar.activation(out=osb, in_=ps_o, func=AF.Relu,
                                         bias=bglob_sb, scale=1.0)
                    nc.sync.dma_start(out=outp.ap()[:, t * TP:(t + 1) * TP],
                                      in_=osb)
                    g_cur = g_nxt
    nc.compile()
    return nc


_NC = None


def _get_nc():
    global _NC
    if _NC is None:
        _NC = _build_kernel()
    return _NC


_PERM = (np.arange(128) + 64) % 128


def _prep_core(core, x, pos, neigh, Wc, Wn, Wr, wd, W_att, W_glob, b_rel, b_glob):
    b = core // 2
    half = core % 2
    P0 = half * NP
    nb = neigh[b][P0:P0 + NP].astype(np.int64)      # [NP, K]

    # packed gather table: row n = [x[:, n] | pos[n] | 0pad]  (bf16)
    tabT = np.zeros((N, 128), dtype=BF)
    tabT[:, 0:64] = x[b].T.astype(BF)
    tabT[:, 64:67] = pos[b].astype(BF)

    # gather idx: tile t cols (k,i) -> nb[t*512+i, k]; wrapped 16 + replicated
    A = nb.reshape(NT, TP, K)                        # [t, i, k]
    V = A.transpose(0, 2, 1).reshape(NT, PKT)        # [t, col] col=k*512+i
    W16 = V.reshape(NT, TP, 16).transpose(0, 2, 1)   # [t, j, s]: idx s*16+j
    idxg = np.tile(W16.transpose(1, 0, 2).reshape(16, NP), (8, 1)).astype(np.int16)

    posCa = pos[b][P0:P0 + NP].T.astype(BF)          # [3, NP]

    w7v = np.zeros((128, 64), dtype=BF)
    w7v[64:67] = (Wn + Wr).astype(BF)
    w7v[67:68] = wd.astype(BF)
    w7v[68:71] = (Wc - Wr).astype(BF)

    w48 = np.zeros((128, 16), dtype=BF)
    for j in range(3):
        for k in range(16):
            w48[16 * j + k, k] = 1

    brel_full = np.zeros((128, 1), np.float32)
    brel_full[64:128, 0] = b_rel

    return {
        "tabT": tabT, "idxg": idxg, "posC": posCa,
        "w7": w7v,
        "watt": W_att[np.ix_(_PERM, _PERM)].astype(BF),
        "wglob": W_glob[_PERM, :].astype(BF),
        "w48": w48,
        "ident": np.eye(128, dtype=BF),
        "brel": brel_full,
        "bglob": b_glob.reshape(128, 1).astype(np.float32),
    }


def kernel(x, pos, neigh_idx, W_rel, b_rel, W_att, W_glob, b_glob, **kw):
    x = np.ascontiguousarray(np.asarray(x, dtype=np.float32))
    pos = np.ascontiguousarray(np.asarray(pos, dtype=np.float32))
    neigh = np.asarray(neigh_idx)
    W_rel = np.asarray(W_rel, dtype=np.float32)
    W_att = np.asarray(W_att, dtype=np.float32)
    W_glob = np.asarray(W_glob, dtype=np.float32)
    b_rel = np.asarray(b_rel, dtype=np.float32)
    b_glob = np.asarray(b_glob, dtype=np.float32)
    Wc, Wn, Wr, wd = W_rel[0:3], W_rel[3:6], W_rel[6:9], W_rel[9:10]

    nc = _get_nc()
    in_maps = [
        _prep_core(core, x, pos, neigh, Wc, Wn, Wr, wd, W_att, W_glob, b_rel, b_glob)
        for core in range(8)
    ]
    res = run_bass_kernel_spmd(nc, in_maps, core_ids=list(range(8)))
    out = np.zeros((B, C_OUT, N), np.float32)
    for core in range(8):
        b = core // 2
        P0 = (core % 2) * NP
        out[b, :, P0:P0 + NP] = res.results[core]["outp"]
    return out


# revision 4
# speedup vs baseline: 1.9065x; 1.1804x over previous
"""Trainium2 Bass kernel for the dense RandLA-Net block.

Reference computation (per batch b, point n, K=16 neighbors):
    enc   = [center(3), npos(3), rel(3), dist(1)]            # 10 dims
    rp    = relu(enc @ W_rel + b_rel)                        # 64
    f     = [rp, nfeat]                                      # 128
    att   = softmax_k(f @ W_att)                             # 128
    agg   = sum_k f * att                                    # 128
    out   = relu(agg @ W_glob + b_glob)                      # 128

Sharding: 8 cores = 4 batches x 2 point-halves (8192 points/core).
Per core the 131072 (point, k) pairs are processed in 16 tiles of 512
points (8192 k-major columns each).  One SWDGE transposed dma_gather per
tile fetches, for each column, a 256-byte row of a packed DRAM table
(bf16: x features at rows 0:64, pos at rows 64:67) straight into the
f-layout [128, 8192] tile.  dist is computed in a k-on-partitions
[48, 512] layout (free-size 512), summed by one matmul, sqrt'd and
DMA'd back into row 67; rp is one 7-row matmul per 512-col chunk
(rel folded: Wc*center + Wn*npos + Wr*(npos-center) = (Wc-Wr)*center +
(Wn+Wr)*npos) whose relu overwrites rows 64:128.  The channel order is
f = [nfeat; rp] everywhere (W_att row+col permuted, W_glob row permuted
host-side).  Softmax-weighted sums over k are PSUM-accumulated identity
matmuls.
"""

import sys

import numpy as np

sys.path.insert(0, "/opt/trn_rl_repo")

import ml_dtypes

import concourse.bass as bass
import concourse.tile as tile
from concourse import mybir, bacc
from concourse.bass_utils import run_bass_kernel_spmd

F32 = mybir.dt.float32
BF16 = mybir.dt.bfloat16
I16 = mybir.dt.int16
AF = mybir.ActivationFunctionType
OP = mybir.AluOpType
BF = ml_dtypes.bfloat16

B, C_IN, N, K = 4, 64, 16384, 16
D_REL, C_MID, C_OUT = 64, 128, 128
NP = N // 2            # points per core
NT = 16                # tiles (point blocks of 512)
TP = NP // NT          # 512 points per tile
PKT = TP * K           # 8192 columns per tile
NCH = 16               # 512-col chunks per tile


def _ap3(t2d, n_idx):
    # [128, n] 2D AP -> [128, 1, n] 3D AP for dma_gather transpose out
    return bass.AP(tensor=t2d.tensor, offset=t2d.offset,
                   ap=[[t2d.ap[0][0], 128], [n_idx, 1], [1, n_idx]])


def _view(t, apl):
    return bass.AP(tensor=t.tensor, offset=t.offset, ap=apl)


def _build_kernel():
    nc = bacc.Bacc("TRN2", target_bir_lowering=False)

    tabT = nc.dram_tensor("tabT", [N, 128], BF16, kind="ExternalInput")
    idxg = nc.dram_tensor("idxg", [128, NP], I16, kind="ExternalInput")
    posC = nc.dram_tensor("posC", [3, NP], BF16, kind="ExternalInput")
    w7 = nc.dram_tensor("w7", [128, 64], BF16, kind="ExternalInput")
    watt = nc.dram_tensor("watt", [128, 128], BF16, kind="ExternalInput")
    wglob = nc.dram_tensor("wglob", [128, 128], BF16, kind="ExternalInput")
    w48 = nc.dram_tensor("w48", [128, 16], BF16, kind="ExternalInput")
    ident = nc.dram_tensor("ident", [128, 128], BF16, kind="ExternalInput")
    brel = nc.dram_tensor("brel", [128, 1], F32, kind="ExternalInput")
    bglob = nc.dram_tensor("bglob", [128, 1], F32, kind="ExternalInput")
    outp = nc.dram_tensor("outp", [128, NP], F32, kind="ExternalOutput")

    with tile.TileContext(nc) as tc:
        with tc.tile_pool(name="persist", bufs=1) as pp:
            idx_sb = pp.tile([128, NP], I16)
            posC_sb = pp.tile([3, NP], BF16)
            cen48 = pp.tile([48, NP], BF16)
            w7_sb = pp.tile([128, 64], BF16)
            watt_sb = pp.tile([128, 128], BF16)
            wglob_sb = pp.tile([128, 128], BF16)
            w48_sb = pp.tile([128, 16], BF16)
            ident_sb = pp.tile([128, 128], BF16)
            brel_sb = pp.tile([128, 1], F32)
            bglob_sb = pp.tile([128, 1], F32)
            nc.sync.dma_start(out=idx_sb, in_=idxg.ap())
            nc.sync.dma_start(out=posC_sb, in_=posC.ap())
            nc.sync.dma_start(out=w7_sb, in_=w7.ap())
            nc.sync.dma_start(out=watt_sb, in_=watt.ap())
            nc.sync.dma_start(out=wglob_sb, in_=wglob.ap())
            nc.sync.dma_start(out=w48_sb, in_=w48.ap())
            nc.sync.dma_start(out=ident_sb, in_=ident.ap())
            nc.sync.dma_start(out=brel_sb, in_=brel.ap())
            nc.sync.dma_start(out=bglob_sb, in_=bglob.ap())
            # cen48[16j+k, p] = posC[j, p]  (center replicated over k)
            src = posC_sb[0:3, :]
            nc.sync.dma_start(
                out=cen48[0:48, :],
                in_=_view(src, [[src.ap[0][0], 3], [0, 16], [1, NP]]))

            with tc.tile_pool(name="gpool", bufs=2) as gp, \
                 tc.tile_pool(name="epool", bufs=2) as ep, \
                 tc.tile_pool(name="spool", bufs=2) as sp, \
                 tc.tile_pool(name="mps", bufs=2, space="PSUM") as mpsum, \
                 tc.tile_pool(name="accps", bufs=1, space="PSUM") as apsum:

                def gather(t):
                    # raw row-gather: g0[p, s, :] = tabT[idx[s*128+p], :]
                    g0 = gp.tile([128, PKT], BF16, tag="g0")
                    g0v = g0[:, :]
                    g03 = _view(g0v, [[g0v.ap[0][0], 128], [128, PKT // 128],
                                      [1, 128]])
                    nc.gpsimd.dma_gather(
                        out_ap=g03, in_ap=tabT.ap(),
                        idxs_ap=idx_sb[:, t * TP:(t + 1) * TP],
                        num_idxs=PKT, num_idxs_reg=PKT, elem_size=128,
                        transpose=False, single_packet=False)
                    # xbar transpose into f-layout: G[c, s*128+p] = g0[p, s*128+c]
                    g = gp.tile([128, PKT], BF16, tag="G")
                    gv = g[:, :]
                    g3 = _view(gv, [[gv.ap[0][0], 128], [128, PKT // 128],
                                    [1, 128]])
                    nc.sync.dma_start_transpose(g3, g0v)
                    return g

                g_cur = gather(0)
                for t in range(NT):
                    g_nxt = gather(t + 1) if t + 1 < NT else None
                    g = g_cur

                    # ---- geometry: dist into G row 67, center into 68:71
                    cb_src = posC_sb[0:3, t * TP:(t + 1) * TP]
                    cb_dst = g[68:71, :]
                    nc.sync.dma_start(
                        out=_view(cb_dst, [[cb_dst.ap[0][0], 3], [TP, 16],
                                           [1, TP]]),
                        in_=_view(cb_src, [[cb_src.ap[0][0], 3], [0, 16],
                                           [1, TP]]))
                    n48 = sp.tile([48, TP], BF16, tag="n48")
                    np_src = g[64:67, :]
                    nc.sync.dma_start(
                        out=n48[0:48, :],
                        in_=_view(np_src, [[np_src.ap[0][0], 3], [TP, 16],
                                           [1, TP]]))
                    nc.vector.tensor_tensor(
                        out=n48, in0=n48,
                        in1=cen48[0:48, t * TP:(t + 1) * TP], op=OP.subtract)
                    nc.vector.tensor_mul(n48, n48, n48)
                    psd = mpsum.tile([128, TP], F32, tag="rp")
                    nc.tensor.matmul(psd[0:16, :], w48_sb[0:48, :],
                                     n48[0:48, :], start=True, stop=True)
                    dsb = sp.tile([16, TP], BF16, tag="dsb")
                    nc.scalar.activation(out=dsb[0:16, :], in_=psd[0:16, :],
                                         func=AF.Sqrt)
                    d_dst = g[67:68, :]
                    nc.sync.dma_start(
                        out=_view(d_dst, [[d_dst.ap[0][0], 1], [TP, 16],
                                          [1, TP]]),
                        in_=dsb[0:16, :])

                    # ---- phase A: rp chunks (PE keeps w7 resident)
                    for cc in range(NCH):
                        cols = slice(cc * TP, (cc + 1) * TP)
                        ps_rp = mpsum.tile([128, TP], F32, tag="rp")
                        nc.tensor.matmul(ps_rp[64:128, :], w7_sb[64:71, :],
                                         g[64:71, cols], start=True, stop=True,
                                         tile_position=(64, 64))
                        if cc % 2 == 0:
                            nc.scalar.activation(out=g[64:128, cols],
                                                 in_=ps_rp[64:128, :],
                                                 func=AF.Relu,
                                                 bias=brel_sb[64:128, :],
                                                 scale=1.0)
                        else:
                            nc.vector.tensor_scalar(out=g[64:128, cols],
                                                    in0=ps_rp[64:128, :],
                                                    scalar1=brel_sb[64:128, :],
                                                    scalar2=0.0,
                                                    op0=OP.add, op1=OP.max)

                    # ---- phase B: attention scores / exp / f*e
                    eu = ep.tile([128, 2 * PKT], BF16, tag="eu")
                    for pr in range(NCH // 2):
                        pcols = slice(pr * 1024, (pr + 1) * 1024)
                        ps_s = mpsum.tile([128, 1024], F32, tag="sc")
                        for ci in range(2):
                            cc = 2 * pr + ci
                            cols = slice(cc * TP, (cc + 1) * TP)
                            nc.tensor.matmul(ps_s[:, ci * TP:(ci + 1) * TP],
                                             watt_sb, g[:, cols],
                                             start=True, stop=True)
                        nc.scalar.activation(out=eu[:, pcols], in_=ps_s,
                                             func=AF.Exp)
                        nc.vector.tensor_mul(
                            eu[:, PKT + pr * 1024:PKT + (pr + 1) * 1024],
                            g[:, pcols], eu[:, pcols])

                    # ---- phase C: accumulate num/den over k
                    ps_den = apsum.tile([128, TP], F32, tag="den")
                    ps_num = apsum.tile([128, TP], F32, tag="num")
                    for cc in range(NCH):
                        nc.tensor.matmul(ps_den, ident_sb,
                                         eu[:, cc * TP:(cc + 1) * TP],
                                         start=(cc == 0), stop=(cc == NCH - 1),
                                         skip_group_check=True)
                        nc.tensor.matmul(ps_num, ident_sb,
                                         eu[:, PKT + cc * TP:PKT + (cc + 1) * TP],
                                         start=(cc == 0), stop=(cc == NCH - 1),
                                         skip_group_check=True)

                    rcp = sp.tile([128, TP], F32, tag="rcp")
                    nc.vector.reciprocal(rcp, ps_den)
                    agg = sp.tile([128, TP], BF16, tag="agg")
                    nc.vector.tensor_mul(agg, ps_num, rcp)
                    ps_o = mpsum.tile([128, TP], F32, tag="rp")
                    nc.tensor.matmul(ps_o, wglob_sb, agg, start=True, stop=True)
                    osb = sp.tile([128, TP], F32, tag="osb")
                    nc.scalar.activation(out=osb, in_=ps_o, func=AF.Relu,
                                         bias=bglob_sb, scale=1.0)
                    nc.sync.dma_start(out=outp.ap()[:, t * TP:(t + 1) * TP],
                                      in_=osb)
                    g_cur = g_nxt
    nc.compile()
    return nc


_NC = None


def _get_nc():
    global _NC
    if _NC is None:
        _NC = _build_kernel()
    return _NC


_PERM = (np.arange(128) + 64) % 128


def _prep_core(core, x, pos, neigh, Wc, Wn, Wr, wd, W_att, W_glob, b_rel, b_glob):
    b = core // 2
    half = core % 2
    P0 = half * NP
    nb = neigh[b][P0:P0 + NP].astype(np.int64)      # [NP, K]

    # packed gather table: row n = [x[:, n] | pos[n] | 0pad]  (bf16)
    tabT = np.zeros((N, 128), dtype=BF)
    tabT[:, 0:64] = x[b].T.astype(BF)
    tabT[:, 64:67] = pos[b].astype(BF)

    # gather idx: tile t cols (k,i) -> nb[t*512+i, k]; wrapped 16 + replicated
    A = nb.reshape(NT, TP, K)                        # [t, i, k]
    V = A.transpose(0, 2, 1).reshape(NT, PKT)        # [t, col] col=k*512+i
    W16 = V.reshape(NT, TP, 16).transpose(0, 2, 1)   # [t, j, s]: idx s*16+j
    idxg = np.tile(W16.transpose(1, 0, 2).reshape(16, NP), (8, 1)).astype(np.int16)

    posCa = pos[b][P0:P0 + NP].T.astype(BF)          # [3, NP]

    w7v = np.zeros((128, 64), dtype=BF)
    w7v[64:67] = (Wn + Wr).astype(BF)
    w7v[67:68] = wd.astype(BF)
    w7v[68:71] = (Wc - Wr).astype(BF)

    w48 = np.zeros((128, 16), dtype=BF)
    for j in range(3):
        for k in range(16):
            w48[16 * j + k, k] = 1

    brel_full = np.zeros((128, 1), np.float32)
    brel_full[64:128, 0] = b_rel

    return {
        "tabT": tabT, "idxg": idxg, "posC": posCa,
        "w7": w7v,
        "watt": W_att[np.ix_(_PERM, _PERM)].astype(BF),
        "wglob": W_glob[_PERM, :].astype(BF),
        "w48": w48,
        "ident": np.eye(128, dtype=BF),
        "brel": brel_full,
        "bglob": b_glob.reshape(128, 1).astype(np.float32),
    }


def kernel(x, pos, neigh_idx, W_rel, b_rel, W_att, W_glob, b_glob, **kw):
    x = np.ascontiguousarray(np.asarray(x, dtype=np.float32))
    pos = np.ascontiguousarray(np.asarray(pos, dtype=np.float32))
    neigh = np.asarray(neigh_idx)
    W_rel = np.asarray(W_rel, dtype=np.float32)
    W_att = np.asarray(W_att, dtype=np.float32)
    W_glob = np.asarray(W_glob, dtype=np.float32)
    b_rel = np.asarray(b_rel, dtype=np.float32)
    b_glob = np.asarray(b_glob, dtype=np.float32)
    Wc, Wn, Wr, wd = W_rel[0:3], W_rel[3:6], W_rel[6:9], W_rel[9:10]

    nc = _get_nc()
    in_maps = [
        _prep_core(core, x, pos, neigh, Wc, Wn, Wr, wd, W_att, W_glob, b_rel, b_glob)
        for core in range(8)
    ]
    res = run_bass_kernel_spmd(nc, in_maps, core_ids=list(range(8)))
    out = np.zeros((B, C_OUT, N), np.float32)
    for core in range(8):
        b = core // 2
        P0 = (core % 2) * NP
        out[b, :, P0:P0 + NP] = res.results[core]["outp"]
    return out


# revision 8
# speedup vs baseline: 3.3705x; 1.7679x over previous
"""Trainium2 Bass kernel for the dense RandLA-Net block.

Reference computation (per batch b, point n, K=16 neighbors):
    enc   = [center(3), npos(3), rel(3), dist(1)]            # 10 dims
    rp    = relu(enc @ W_rel + b_rel)                        # 64
    f     = [rp, nfeat]                                      # 128
    att   = softmax_k(f @ W_att)                             # 128
    agg   = sum_k f * att                                    # 128
    out   = relu(agg @ W_glob + b_glob)                      # 128

Sharding: 8 cores = 4 batches x 2 point-halves (8192 points/core).
Per core the 131072 (point, k) pairs are processed in 16 tiles of 512
points (8192 k-major columns each).  One SWDGE transposed dma_gather per
tile fetches, for each column, a 256-byte row of a packed DRAM table
(bf16: x features at rows 0:64, pos at rows 64:67) straight into the
f-layout [128, 8192] tile.  dist is computed in a k-on-partitions
[48, 512] layout (free-size 512), summed by one matmul, sqrt'd and
DMA'd back into row 67; rp is one 7-row matmul per 512-col chunk
(rel folded: Wc*center + Wn*npos + Wr*(npos-center) = (Wc-Wr)*center +
(Wn+Wr)*npos) whose relu overwrites rows 64:128.  The channel order is
f = [nfeat; rp] everywhere (W_att row+col permuted, W_glob row permuted
host-side).  Softmax-weighted sums over k are PSUM-accumulated identity
matmuls.
"""

import sys

import numpy as np

sys.path.insert(0, "/opt/trn_rl_repo")

import ml_dtypes

import concourse.bass as bass
import concourse.tile as tile
from concourse import mybir, bacc
from concourse.bass_utils import run_bass_kernel_spmd

F32 = mybir.dt.float32
BF16 = mybir.dt.bfloat16
I16 = mybir.dt.int16
AF = mybir.ActivationFunctionType
OP = mybir.AluOpType
BF = ml_dtypes.bfloat16

B, C_IN, N, K = 4, 64, 16384, 16
D_REL, C_MID, C_OUT = 64, 128, 128
NP = N // 2            # points per core
NT = 16                # tiles (point blocks of 512)
TP = NP // NT          # 512 points per tile
PKT = TP * K           # 8192 columns per tile
NCH = 16               # 512-col chunks per tile


def _ap3(t2d, n_idx):
    # [128, n] 2D AP -> [128, 1, n] 3D AP for dma_gather transpose out
    return bass.AP(tensor=t2d.tensor, offset=t2d.offset,
                   ap=[[t2d.ap[0][0], 128], [n_idx, 1], [1, n_idx]])


def _view(t, apl):
    return bass.AP(tensor=t.tensor, offset=t.offset, ap=apl)


def _build_kernel():
    nc = bacc.Bacc("TRN2", target_bir_lowering=False, num_swdge_queues=4)

    tabT = nc.dram_tensor("tabT", [N, 128], BF16, kind="ExternalInput")
    idxg = nc.dram_tensor("idxg", [128, NP], I16, kind="ExternalInput")
    posC = nc.dram_tensor("posC", [3, NP], BF16, kind="ExternalInput")
    w7 = nc.dram_tensor("w7", [128, 64], BF16, kind="ExternalInput")
    watt = nc.dram_tensor("watt", [128, 128], BF16, kind="ExternalInput")
    wglob = nc.dram_tensor("wglob", [128, 128], BF16, kind="ExternalInput")
    w48 = nc.dram_tensor("w48", [128, 16], BF16, kind="ExternalInput")
    ident = nc.dram_tensor("ident", [128, 128], BF16, kind="ExternalInput")
    brel = nc.dram_tensor("brel", [128, 1], F32, kind="ExternalInput")
    bglob = nc.dram_tensor("bglob", [128, 1], F32, kind="ExternalInput")
    outp = nc.dram_tensor("outp", [128, NP], F32, kind="ExternalOutput")

    with tile.TileContext(nc) as tc:
        with tc.tile_pool(name="persist", bufs=1) as pp:
            idx_sb = pp.tile([128, NP], I16)
            posC_sb = pp.tile([3, NP], BF16)
            cen48 = pp.tile([48, NP], BF16)
            w7_sb = pp.tile([128, 64], BF16)
            watt_sb = pp.tile([128, 128], BF16)
            wglob_sb = pp.tile([128, 128], BF16)
            w48_sb = pp.tile([128, 16], BF16)
            ident_sb = pp.tile([128, 128], BF16)
            brel_sb = pp.tile([128, 1], F32)
            bglob_sb = pp.tile([128, 1], F32)
            nc.sync.dma_start(out=idx_sb, in_=idxg.ap())
            nc.sync.dma_start(out=posC_sb, in_=posC.ap())
            nc.sync.dma_start(out=w7_sb, in_=w7.ap())
            nc.sync.dma_start(out=watt_sb, in_=watt.ap())
            nc.sync.dma_start(out=wglob_sb, in_=wglob.ap())
            nc.sync.dma_start(out=w48_sb, in_=w48.ap())
            nc.sync.dma_start(out=ident_sb, in_=ident.ap())
            nc.sync.dma_start(out=brel_sb, in_=brel.ap())
            nc.sync.dma_start(out=bglob_sb, in_=bglob.ap())
            # cen48[16j+k, p] = posC[j, p]  (center replicated over k)
            src = posC_sb[0:3, :]
            nc.sync.dma_start(
                out=cen48[0:48, :],
                in_=_view(src, [[src.ap[0][0], 3], [0, 16], [1, NP]]))

            with tc.tile_pool(name="gpool", bufs=2) as gp, \
                 tc.tile_pool(name="epool", bufs=2) as ep, \
                 tc.tile_pool(name="spool", bufs=2) as sp, \
                 tc.tile_pool(name="mps", bufs=2, space="PSUM") as mpsum, \
                 tc.tile_pool(name="accps", bufs=1, space="PSUM") as apsum:

                def gather(t):
                    # raw row-gather: g0[p, s, :] = tabT[idx[s*128+p], :]
                    # split over the 4 SWDGE queues so the rings drain in
                    # parallel (each ring is serviced at ~1 DMA engine rate)
                    g0 = gp.tile([128, PKT], BF16, tag="g0")
                    NQ, SUB = 4, PKT // 4
                    for q in range(NQ):
                        g0v = g0[:, q * SUB:(q + 1) * SUB]
                        g03 = _view(g0v, [[g0v.ap[0][0], 128],
                                          [128, SUB // 128], [1, 128]])
                        nc.gpsimd.dma_gather(
                            out_ap=g03, in_ap=tabT.ap(),
                            idxs_ap=idx_sb[:, t * TP + q * (SUB // 16):
                                           t * TP + (q + 1) * (SUB // 16)],
                            num_idxs=SUB, num_idxs_reg=SUB, elem_size=128,
                            transpose=False, single_packet=False, queue_num=q)
                    # xbar transpose into f-layout: G[c, s*128+p] = g0[p, s*128+c]
                    g = gp.tile([128, PKT], BF16, tag="G")
                    gv = g[:, :]
                    g3 = _view(gv, [[gv.ap[0][0], 128], [128, PKT // 128],
                                    [1, 128]])
                    nc.sync.dma_start_transpose(g3, g0[:, :])
                    return g

                g_cur = gather(0)
                for t in range(NT):
                    g_nxt = gather(t + 1) if t + 1 < NT else None
                    g = g_cur

                    # ---- geometry: dist into G row 67, center into 68:71
                    cb_src = posC_sb[0:3, t * TP:(t + 1) * TP]
                    cb_dst = g[68:71, :]
                    nc.sync.dma_start(
                        out=_view(cb_dst, [[cb_dst.ap[0][0], 3], [TP, 16],
                                           [1, TP]]),
                        in_=_view(cb_src, [[cb_src.ap[0][0], 3], [0, 16],
                                           [1, TP]]))
                    n48 = sp.tile([48, TP], BF16, tag="n48")
                    np_src = g[64:67, :]
                    nc.sync.dma_start(
                        out=n48[0:48, :],
                        in_=_view(np_src, [[np_src.ap[0][0], 3], [TP, 16],
                                           [1, TP]]))
                    nc.vector.tensor_tensor(
                        out=n48, in0=n48,
                        in1=cen48[0:48, t * TP:(t + 1) * TP], op=OP.subtract)
                    nc.vector.tensor_mul(n48, n48, n48)
                    psd = mpsum.tile([128, TP], F32, tag="rp")
                    nc.tensor.matmul(psd[0:16, :], w48_sb[0:48, :],
                                     n48[0:48, :], start=True, stop=True)
                    dsb = sp.tile([16, TP], BF16, tag="dsb")
                    nc.scalar.activation(out=dsb[0:16, :], in_=psd[0:16, :],
                                         func=AF.Sqrt)
                    d_dst = g[67:68, :]
                    nc.sync.dma_start(
                        out=_view(d_dst, [[d_dst.ap[0][0], 1], [TP, 16],
                                          [1, TP]]),
                        in_=dsb[0:16, :])

                    # ---- phase A: rp chunks (PE keeps w7 resident)
                    for cc in range(NCH):
                        cols = slice(cc * TP, (cc + 1) * TP)
                        ps_rp = mpsum.tile([128, TP], F32, tag="rp")
                        nc.tensor.matmul(ps_rp[64:128, :], w7_sb[64:71, :],
                                         g[64:71, cols], start=True, stop=True,
                                         tile_position=(64, 64))
                        if cc % 2 == 0:
                            nc.scalar.activation(out=g[64:128, cols],
                                                 in_=ps_rp[64:128, :],
                                                 func=AF.Relu,
                                                 bias=brel_sb[64:128, :],
                                                 scale=1.0)
                        else:
                            nc.vector.tensor_scalar(out=g[64:128, cols],
                                                    in0=ps_rp[64:128, :],
                                                    scalar1=brel_sb[64:128, :],
                                                    scalar2=0.0,
                                                    op0=OP.add, op1=OP.max)

                    # ---- phase B: attention scores / exp / f*e
                    eu = ep.tile([128, 2 * PKT], BF16, tag="eu")
                    for pr in range(NCH // 2):
                        pcols = slice(pr * 1024, (pr + 1) * 1024)
                        ps_s = mpsum.tile([128, 1024], F32, tag="sc")
                        for ci in range(2):
                            cc = 2 * pr + ci
                            cols = slice(cc * TP, (cc + 1) * TP)
                            nc.tensor.matmul(ps_s[:, ci * TP:(ci + 1) * TP],
                                             watt_sb, g[:, cols],
                                             start=True, stop=True)
                        nc.scalar.activation(out=eu[:, pcols], in_=ps_s,
                                             func=AF.Exp)
                        nc.vector.tensor_mul(
                            eu[:, PKT + pr * 1024:PKT + (pr + 1) * 1024],
                            g[:, pcols], eu[:, pcols])

                    # ---- phase C: accumulate num/den over k
                    ps_den = apsum.tile([128, TP], F32, tag="den")
                    ps_num = apsum.tile([128, TP], F32, tag="num")
                    for cc in range(NCH):
                        nc.tensor.matmul(ps_den, ident_sb,
                                         eu[:, cc * TP:(cc + 1) * TP],
                                         start=(cc == 0), stop=(cc == NCH - 1),
                                         skip_group_check=True)
                        nc.tensor.matmul(ps_num, ident_sb,
                                         eu[:, PKT + cc * TP:PKT + (cc + 1) * TP],
                                         start=(cc == 0), stop=(cc == NCH - 1),
                                         skip_group_check=True)

                    rcp = sp.tile([128, TP], F32, tag="rcp")
                    nc.vector.reciprocal(rcp, ps_den)
                    agg = sp.tile([128, TP], BF16, tag="agg")
                    nc.vector.tensor_mul(agg, ps_num, rcp)
                    ps_o = mpsum.tile([128, TP], F32, tag="rp")
                    nc.tensor.matmul(ps_o, wglob_sb, agg, start=True, stop=True)
                    osb = sp.tile([128, TP], F32, tag="osb")
                    nc.scalar.activation(out=osb, in_=ps_o, func=AF.Relu,
                                         bias=bglob_sb, scale=1.0)
                    nc.sync.dma_start(out=outp.ap()[:, t * TP:(t + 1) * TP],
                                      in_=osb)
                    g_cur = g_nxt
    nc.compile()
    return nc


_NC = None


def _get_nc():
    global _NC
    if _NC is None:
        _NC = _build_kernel()
    return _NC


_PERM = (np.arange(128) + 64) % 128


def _prep_core(core, x, pos, neigh, Wc, Wn, Wr, wd, W_att, W_glob, b_rel, b_glob):
    b = core // 2
    half = core % 2
    P0 = half * NP
    nb = neigh[b][P0:P0 + NP].astype(np.int64)      # [NP, K]

    # packed gather table: row n = [x[:, n] | pos[n] | 0pad]  (bf16)
    tabT = np.zeros((N, 128), dtype=BF)
    tabT[:, 0:64] = x[b].T.astype(BF)
    tabT[:, 64:67] = pos[b].astype(BF)

    # gather idx: tile t cols (k,i) -> nb[t*512+i, k]; wrapped 16 + replicated
    A = nb.reshape(NT, TP, K)                        # [t, i, k]
    V = A.transpose(0, 2, 1).reshape(NT, PKT)        # [t, col] col=k*512+i
    W16 = V.reshape(NT, TP, 16).transpose(0, 2, 1)   # [t, j, s]: idx s*16+j
    idxg = np.tile(W16.transpose(1, 0, 2).reshape(16, NP), (8, 1)).astype(np.int16)

    posCa = pos[b][P0:P0 + NP].T.astype(BF)          # [3, NP]

    w7v = np.zeros((128, 64), dtype=BF)
    w7v[64:67] = (Wn + Wr).astype(BF)
    w7v[67:68] = wd.astype(BF)
    w7v[68:71] = (Wc - Wr).astype(BF)

    w48 = np.zeros((128, 16), dtype=BF)
    for j in range(3):
        for k in range(16):
            w48[16 * j + k, k] = 1

    brel_full = np.zeros((128, 1), np.float32)
    brel_full[64:128, 0] = b_rel

    return {
        "tabT": tabT, "idxg": idxg, "posC": posCa,
        "w7": w7v,
        "watt": W_att[np.ix_(_PERM, _PERM)].astype(BF),
        "wglob": W_glob[_PERM, :].astype(BF),
        "w48": w48,
        "ident": np.eye(128, dtype=BF),
        "brel": brel_full,
        "bglob": b_glob.reshape(128, 1).astype(np.float32),
    }


def kernel(x, pos, neigh_idx, W_rel, b_rel, W_att, W_glob, b_glob, **kw):
    x = np.ascontiguousarray(np.asarray(x, dtype=np.float32))
    pos = np.ascontiguousarray(np.asarray(pos, dtype=np.float32))
    neigh = np.asarray(neigh_idx)
    W_rel = np.asarray(W_rel, dtype=np.float32)
    W_att = np.asarray(W_att, dtype=np.float32)
    W_glob = np.asarray(W_glob, dtype=np.float32)
    b_rel = np.asarray(b_rel, dtype=np.float32)
    b_glob = np.asarray(b_glob, dtype=np.float32)
    Wc, Wn, Wr, wd = W_rel[0:3], W_rel[3:6], W_rel[6:9], W_rel[9:10]

    nc = _get_nc()
    in_maps = [
        _prep_core(core, x, pos, neigh, Wc, Wn, Wr, wd, W_att, W_glob, b_rel, b_glob)
        for core in range(8)
    ]
    res = run_bass_kernel_spmd(nc, in_maps, core_ids=list(range(8)))
    out = np.zeros((B, C_OUT, N), np.float32)
    for core in range(8):
        b = core // 2
        P0 = (core % 2) * NP
        out[b, :, P0:P0 + NP] = res.results[core]["outp"]
    return out


# revision 11
# speedup vs baseline: 3.8457x; 1.1410x over previous
"""Trainium2 Bass kernel for the dense RandLA-Net block.

Reference computation (per batch b, point n, K=16 neighbors):
    enc   = [center(3), npos(3), rel(3), dist(1)]            # 10 dims
    rp    = relu(enc @ W_rel + b_rel)                        # 64
    f     = [rp, nfeat]                                      # 128
    att   = softmax_k(f @ W_att)                             # 128
    agg   = sum_k f * att                                    # 128
    out   = relu(agg @ W_glob + b_glob)                      # 128

Sharding: 8 cores = 4 batches x 2 point-halves (8192 points/core).
Per core the 131072 (point, k) pairs are processed in 16 tiles of 512
points (8192 k-major columns each).  One SWDGE transposed dma_gather per
tile fetches, for each column, a 256-byte row of a packed DRAM table
(bf16: x features at rows 0:64, pos at rows 64:67) straight into the
f-layout [128, 8192] tile.  dist is computed in a k-on-partitions
[48, 512] layout (free-size 512), summed by one matmul, sqrt'd and
DMA'd back into row 67; rp is one 7-row matmul per 512-col chunk
(rel folded: Wc*center + Wn*npos + Wr*(npos-center) = (Wc-Wr)*center +
(Wn+Wr)*npos) whose relu overwrites rows 64:128.  The channel order is
f = [nfeat; rp] everywhere (W_att row+col permuted, W_glob row permuted
host-side).  Softmax-weighted sums over k are PSUM-accumulated identity
matmuls.
"""

import sys

import numpy as np

sys.path.insert(0, "/opt/trn_rl_repo")

import ml_dtypes

import concourse.bass as bass
import concourse.tile as tile
from concourse import mybir, bacc
from concourse.bass_utils import run_bass_kernel_spmd

F32 = mybir.dt.float32
BF16 = mybir.dt.bfloat16
I16 = mybir.dt.int16
AF = mybir.ActivationFunctionType
OP = mybir.AluOpType
BF = ml_dtypes.bfloat16

B, C_IN, N, K = 4, 64, 16384, 16
D_REL, C_MID, C_OUT = 64, 128, 128
NP = N // 2            # points per core
NT = 16                # tiles (point blocks of 512)
TP = NP // NT          # 512 points per tile
PKT = TP * K           # 8192 columns per tile
NCH = 16               # 512-col chunks per tile


def _ap3(t2d, n_idx):
    # [128, n] 2D AP -> [128, 1, n] 3D AP for dma_gather transpose out
    return bass.AP(tensor=t2d.tensor, offset=t2d.offset,
                   ap=[[t2d.ap[0][0], 128], [n_idx, 1], [1, n_idx]])


def _view(t, apl):
    return bass.AP(tensor=t.tensor, offset=t.offset, ap=apl)


def _build_kernel():
    nc = bacc.Bacc("TRN2", target_bir_lowering=False, num_swdge_queues=4)

    tabT = nc.dram_tensor("tabT", [N, 128], BF16, kind="ExternalInput")
    idxg = nc.dram_tensor("idxg", [128, NP], I16, kind="ExternalInput")
    posC = nc.dram_tensor("posC", [3, NP], BF16, kind="ExternalInput")
    w7 = nc.dram_tensor("w7", [128, 64], BF16, kind="ExternalInput")
    watt = nc.dram_tensor("watt", [128, 128], BF16, kind="ExternalInput")
    wglob = nc.dram_tensor("wglob", [128, 128], BF16, kind="ExternalInput")
    w48 = nc.dram_tensor("w48", [128, 16], BF16, kind="ExternalInput")
    ident = nc.dram_tensor("ident", [128, 128], BF16, kind="ExternalInput")
    brel = nc.dram_tensor("brel", [128, 1], F32, kind="ExternalInput")
    bglob = nc.dram_tensor("bglob", [128, 1], F32, kind="ExternalInput")
    outp = nc.dram_tensor("outp", [128, NP], F32, kind="ExternalOutput")

    with tile.TileContext(nc) as tc:
        with tc.tile_pool(name="persist", bufs=1) as pp:
            idx_sb = pp.tile([128, NP], I16)
            posC_sb = pp.tile([3, NP], BF16)
            cen48 = pp.tile([48, NP], BF16)
            w7_sb = pp.tile([128, 64], BF16)
            watt_sb = pp.tile([128, 128], BF16)
            wglob_sb = pp.tile([128, 128], BF16)
            w48_sb = pp.tile([128, 16], BF16)
            ident_sb = pp.tile([128, 128], BF16)
            brel_sb = pp.tile([128, 1], F32)
            bglob_sb = pp.tile([128, 1], F32)
            nc.sync.dma_start(out=idx_sb, in_=idxg.ap())
            nc.sync.dma_start(out=posC_sb, in_=posC.ap())
            nc.sync.dma_start(out=w7_sb, in_=w7.ap())
            nc.sync.dma_start(out=watt_sb, in_=watt.ap())
            nc.sync.dma_start(out=wglob_sb, in_=wglob.ap())
            nc.sync.dma_start(out=w48_sb, in_=w48.ap())
            nc.sync.dma_start(out=ident_sb, in_=ident.ap())
            nc.sync.dma_start(out=brel_sb, in_=brel.ap())
            nc.sync.dma_start(out=bglob_sb, in_=bglob.ap())
            # cen48[16j+k, p] = posC[j, p]  (center replicated over k)
            src = posC_sb[0:3, :]
            nc.sync.dma_start(
                out=cen48[0:48, :],
                in_=_view(src, [[src.ap[0][0], 3], [0, 16], [1, NP]]))

            with tc.tile_pool(name="g0pool", bufs=2) as g0p, \
                 tc.tile_pool(name="gpool", bufs=3) as gp, \
                 tc.tile_pool(name="epool", bufs=2) as ep, \
                 tc.tile_pool(name="spool", bufs=2) as sp, \
                 tc.tile_pool(name="mps", bufs=2, space="PSUM") as mpsum, \
                 tc.tile_pool(name="accps", bufs=1, space="PSUM") as apsum:

                def gather(t):
                    # raw row-gather: g0[p, s, :] = tabT[idx[s*128+p], :]
                    # split over the 4 SWDGE queues so the rings drain in
                    # parallel (each ring is serviced at ~1 DMA engine rate)
                    g0 = g0p.tile([128, PKT], BF16, tag="g0")
                    NQ, SUB = 4, PKT // 4
                    for q in range(NQ):
                        g0v = g0[:, q * SUB:(q + 1) * SUB]
                        g03 = _view(g0v, [[g0v.ap[0][0], 128],
                                          [128, SUB // 128], [1, 128]])
                        nc.gpsimd.dma_gather(
                            out_ap=g03, in_ap=tabT.ap(),
                            idxs_ap=idx_sb[:, t * TP + q * (SUB // 16):
                                           t * TP + (q + 1) * (SUB // 16)],
                            num_idxs=SUB, num_idxs_reg=SUB, elem_size=128,
                            transpose=False, single_packet=False, queue_num=q)
                    return g0

                def transpose_geom(t, g0):
                    # xbar transpose into f-layout: G[c, s*128+p] = g0[p, s*128+c]
                    g = gp.tile([128, PKT], BF16, tag="G")
                    gv = g[:, :]
                    g3 = _view(gv, [[gv.ap[0][0], 128], [128, PKT // 128],
                                    [1, 128]])
                    nc.sync.dma_start_transpose(g3, g0[:, :])

                    # geometry: dist into G row 67, center into 68:71
                    cb_src = posC_sb[0:3, t * TP:(t + 1) * TP]
                    cb_dst = g[68:71, :]
                    nc.sync.dma_start(
                        out=_view(cb_dst, [[cb_dst.ap[0][0], 3], [TP, 16],
                                           [1, TP]]),
                        in_=_view(cb_src, [[cb_src.ap[0][0], 3], [0, 16],
                                           [1, TP]]))
                    n48 = sp.tile([48, TP], BF16, tag="n48")
                    np_src = g[64:67, :]
                    nc.sync.dma_start(
                        out=n48[0:48, :],
                        in_=_view(np_src, [[np_src.ap[0][0], 3], [TP, 16],
                                           [1, TP]]))
                    nc.vector.tensor_tensor(
                        out=n48, in0=n48,
                        in1=cen48[0:48, t * TP:(t + 1) * TP], op=OP.subtract)
                    nc.vector.tensor_mul(n48, n48, n48)
                    psd = mpsum.tile([128, TP], F32, tag="rp")
                    nc.tensor.matmul(psd[0:16, :], w48_sb[0:48, :],
                                     n48[0:48, :], start=True, stop=True)
                    dsb = sp.tile([16, TP], BF16, tag="dsb")
                    nc.scalar.activation(out=dsb[0:16, :], in_=psd[0:16, :],
                                         func=AF.Sqrt)
                    d_dst = g[67:68, :]
                    nc.sync.dma_start(
                        out=_view(d_dst, [[d_dst.ap[0][0], 1], [TP, 16],
                                          [1, TP]]),
                        in_=dsb[0:16, :])
                    return g

                g0_list = [gather(0), gather(1)]
                g_list = [transpose_geom(0, g0_list[0])]
                for t in range(NT):
                    if t + 2 < NT:
                        g0_list.append(gather(t + 2))
                    if t + 1 < NT:
                        g_list.append(transpose_geom(t + 1, g0_list[t + 1]))
                    g = g_list[t]

                    # ---- phase A: rp chunks (PE keeps w7 resident)
                    for cc in range(NCH):
                        cols = slice(cc * TP, (cc + 1) * TP)
                        ps_rp = mpsum.tile([128, TP], F32, tag="rp")
                        nc.tensor.matmul(ps_rp[64:128, :], w7_sb[64:71, :],
                                         g[64:71, cols], start=True, stop=True,
                                         tile_position=(64, 64))
                        if cc % 2 == 0:
                            nc.scalar.activation(out=g[64:128, cols],
                                                 in_=ps_rp[64:128, :],
                                                 func=AF.Relu,
                                                 bias=brel_sb[64:128, :],
                                                 scale=1.0)
                        else:
                            nc.vector.tensor_scalar(out=g[64:128, cols],
                                                    in0=ps_rp[64:128, :],
                                                    scalar1=brel_sb[64:128, :],
                                                    scalar2=0.0,
                                                    op0=OP.add, op1=OP.max)

                    # ---- phase B: attention scores / exp / f*e (f*e in-place in G)
                    eu = ep.tile([128, PKT], BF16, tag="eu")
                    for pr in range(NCH // 2):
                        pcols = slice(pr * 1024, (pr + 1) * 1024)
                        ps_s = mpsum.tile([128, 1024], F32, tag="sc")
                        for ci in range(2):
                            cc = 2 * pr + ci
                            cols = slice(cc * TP, (cc + 1) * TP)
                            nc.tensor.matmul(ps_s[:, ci * TP:(ci + 1) * TP],
                                             watt_sb, g[:, cols],
                                             start=True, stop=True)
                        nc.scalar.activation(out=eu[:, pcols], in_=ps_s,
                                             func=AF.Exp)
                        nc.vector.tensor_mul(g[:, pcols], g[:, pcols],
                                             eu[:, pcols])

                    # ---- phase C: accumulate num/den over k
                    ps_den = apsum.tile([128, TP], F32, tag="den")
                    ps_num = apsum.tile([128, TP], F32, tag="num")
                    for cc in range(NCH):
                        nc.tensor.matmul(ps_den, ident_sb,
                                         eu[:, cc * TP:(cc + 1) * TP],
                                         start=(cc == 0), stop=(cc == NCH - 1),
                                         skip_group_check=True)
                        nc.tensor.matmul(ps_num, ident_sb,
                                         g[:, cc * TP:(cc + 1) * TP],
                                         start=(cc == 0), stop=(cc == NCH - 1),
                                         skip_group_check=True)

                    rcp = sp.tile([128, TP], F32, tag="rcp")
                    nc.vector.reciprocal(rcp, ps_den)
                    agg = sp.tile([128, TP], BF16, tag="agg")
                    nc.vector.tensor_mul(agg, ps_num, rcp)
                    ps_o = mpsum.tile([128, TP], F32, tag="rp")
                    nc.tensor.matmul(ps_o, wglob_sb, agg, start=True, stop=True)
                    osb = sp.tile([128, TP], F32, tag="osb")
                    nc.scalar.activation(out=osb, in_=ps_o, func=AF.Relu,
                                         bias=bglob_sb, scale=1.0)
                    nc.sync.dma_start(out=outp.ap()[:, t * TP:(t + 1) * TP],
                                      in_=osb)
    nc.compile()
    return nc


_NC = None


def _get_nc():
    global _NC
    if _NC is None:
        _NC = _build_kernel()
    return _NC


_PERM = (np.arange(128) + 64) % 128


def _prep_core(core, x, pos, neigh, Wc, Wn, Wr, wd, W_att, W_glob, b_rel, b_glob):
    b = core // 2
    half = core % 2
    P0 = half * NP
    nb = neigh[b][P0:P0 + NP].astype(np.int64)      # [NP, K]

    # packed gather table: row n = [x[:, n] | pos[n] | 0pad]  (bf16)
    tabT = np.zeros((N, 128), dtype=BF)
    tabT[:, 0:64] = x[b].T.astype(BF)
    tabT[:, 64:67] = pos[b].astype(BF)

    # gather idx: tile t cols (k,i) -> nb[t*512+i, k]; wrapped 16 + replicated
    A = nb.reshape(NT, TP, K)                        # [t, i, k]
    V = A.transpose(0, 2, 1).reshape(NT, PKT)        # [t, col] col=k*512+i
    W16 = V.reshape(NT, TP, 16).transpose(0, 2, 1)   # [t, j, s]: idx s*16+j
    idxg = np.tile(W16.transpose(1, 0, 2).reshape(16, NP), (8, 1)).astype(np.int16)

    posCa = pos[b][P0:P0 + NP].T.astype(BF)          # [3, NP]

    w7v = np.zeros((128, 64), dtype=BF)
    w7v[64:67] = (Wn + Wr).astype(BF)
    w7v[67:68] = wd.astype(BF)
    w7v[68:71] = (Wc - Wr).astype(BF)

    w48 = np.zeros((128, 16), dtype=BF)
    for j in range(3):
        for k in range(16):
            w48[16 * j + k, k] = 1

    brel_full = np.zeros((128, 1), np.float32)
    brel_full[64:128, 0] = b_rel

    return {
        "tabT": tabT, "idxg": idxg, "posC": posCa,
        "w7": w7v,
        "watt": W_att[np.ix_(_PERM, _PERM)].astype(BF),
        "wglob": W_glob[_PERM, :].astype(BF),
        "w48": w48,
        "ident": np.eye(128, dtype=BF),
        "brel": brel_full,
        "bglob": b_glob.reshape(128, 1).astype(np.float32),
    }


def kernel(x, pos, neigh_idx, W_rel, b_rel, W_att, W_glob, b_glob, **kw):
    x = np.ascontiguousarray(np.asarray(x, dtype=np.float32))
    pos = np.ascontiguousarray(np.asarray(pos, dtype=np.float32))
    neigh = np.asarray(neigh_idx)
    W_rel = np.asarray(W_rel, dtype=np.float32)
    W_att = np.asarray(W_att, dtype=np.float32)
    W_glob = np.asarray(W_glob, dtype=np.float32)
    b_rel = np.asarray(b_rel, dtype=np.float32)
    b_glob = np.asarray(b_glob, dtype=np.float32)
    Wc, Wn, Wr, wd = W_rel[0:3], W_rel[3:6], W_rel[6:9], W_rel[9:10]

    nc = _get_nc()
    in_maps = [
        _prep_core(core, x, pos, neigh, Wc, Wn, Wr, wd, W_att, W_glob, b_rel, b_glob)
        for core in range(8)
    ]
    res = run_bass_kernel_spmd(nc, in_maps, core_ids=list(range(8)))
    out = np.zeros((B, C_OUT, N), np.float32)
    for core in range(8):
        b = core // 2
        P0 = (core % 2) * NP
        out[b, :, P0:P0 + NP] = res.results[core]["outp"]
    return out


# revision 14
# speedup vs baseline: 3.8875x; 1.0109x over previous
"""Trainium2 Bass kernel for the dense RandLA-Net block.

Reference computation (per batch b, point n, K=16 neighbors):
    enc   = [center(3), npos(3), rel(3), dist(1)]            # 10 dims
    rp    = relu(enc @ W_rel + b_rel)                        # 64
    f     = [rp, nfeat]                                      # 128
    att   = softmax_k(f @ W_att)                             # 128
    agg   = sum_k f * att                                    # 128
    out   = relu(agg @ W_glob + b_glob)                      # 128

Sharding: 8 cores = 4 batches x 2 point-halves (8192 points/core).
Per core the 131072 (point, k) pairs are processed in 16 tiles of 512
points (8192 k-major columns each).  One SWDGE transposed dma_gather per
tile fetches, for each column, a 256-byte row of a packed DRAM table
(bf16: x features at rows 0:64, pos at rows 64:67) straight into the
f-layout [128, 8192] tile.  dist is computed in a k-on-partitions
[48, 512] layout (free-size 512), summed by one matmul, sqrt'd and
DMA'd back into row 67; rp is one 7-row matmul per 512-col chunk
(rel folded: Wc*center + Wn*npos + Wr*(npos-center) = (Wc-Wr)*center +
(Wn+Wr)*npos) whose relu overwrites rows 64:128.  The channel order is
f = [nfeat; rp] everywhere (W_att row+col permuted, W_glob row permuted
host-side).  Softmax-weighted sums over k are PSUM-accumulated identity
matmuls.
"""

import sys

import numpy as np

sys.path.insert(0, "/opt/trn_rl_repo")

import ml_dtypes

import concourse.bass as bass
import concourse.tile as tile
from concourse import mybir, bacc
from concourse.bass_utils import run_bass_kernel_spmd

F32 = mybir.dt.float32
BF16 = mybir.dt.bfloat16
I16 = mybir.dt.int16
AF = mybir.ActivationFunctionType
OP = mybir.AluOpType
BF = ml_dtypes.bfloat16

B, C_IN, N, K = 4, 64, 16384, 16
D_REL, C_MID, C_OUT = 64, 128, 128
NP = N // 2            # points per core
NT = 16                # tiles (point blocks of 512)
TP = NP // NT          # 512 points per tile
PKT = TP * K           # 8192 columns per tile
NCH = 16               # 512-col chunks per tile


def _ap3(t2d, n_idx):
    # [128, n] 2D AP -> [128, 1, n] 3D AP for dma_gather transpose out
    return bass.AP(tensor=t2d.tensor, offset=t2d.offset,
                   ap=[[t2d.ap[0][0], 128], [n_idx, 1], [1, n_idx]])


def _view(t, apl):
    return bass.AP(tensor=t.tensor, offset=t.offset, ap=apl)


def _build_kernel():
    nc = bacc.Bacc("TRN2", target_bir_lowering=False, num_swdge_queues=4)

    tabT = nc.dram_tensor("tabT", [N, 128], BF16, kind="ExternalInput")
    idxg = nc.dram_tensor("idxg", [128, NP], I16, kind="ExternalInput")
    posC = nc.dram_tensor("posC", [3, NP], BF16, kind="ExternalInput")
    w7 = nc.dram_tensor("w7", [128, 64], BF16, kind="ExternalInput")
    watt = nc.dram_tensor("watt", [128, 128], BF16, kind="ExternalInput")
    wglob = nc.dram_tensor("wglob", [128, 128], BF16, kind="ExternalInput")
    w48 = nc.dram_tensor("w48", [128, 16], BF16, kind="ExternalInput")
    ident = nc.dram_tensor("ident", [128, 128], BF16, kind="ExternalInput")
    brel = nc.dram_tensor("brel", [128, 1], F32, kind="ExternalInput")
    bglob = nc.dram_tensor("bglob", [128, 1], F32, kind="ExternalInput")
    outp = nc.dram_tensor("outp", [128, NP], F32, kind="ExternalOutput")

    with tile.TileContext(nc) as tc:
        with tc.tile_pool(name="persist", bufs=1) as pp:
            idx_sb = pp.tile([128, NP], I16)
            posC_sb = pp.tile([3, NP], BF16)
            cen48 = pp.tile([48, NP], BF16)
            w7_sb = pp.tile([128, 64], BF16)
            watt_sb = pp.tile([128, 128], BF16)
            wglob_sb = pp.tile([128, 128], BF16)
            w48_sb = pp.tile([128, 16], BF16)
            ident_sb = pp.tile([128, 128], BF16)
            brel_sb = pp.tile([128, 1], F32)
            bglob_sb = pp.tile([128, 1], F32)
            nc.sync.dma_start(out=idx_sb, in_=idxg.ap())
            nc.sync.dma_start(out=posC_sb, in_=posC.ap())
            nc.sync.dma_start(out=w7_sb, in_=w7.ap())
            nc.sync.dma_start(out=watt_sb, in_=watt.ap())
            nc.sync.dma_start(out=wglob_sb, in_=wglob.ap())
            nc.sync.dma_start(out=w48_sb, in_=w48.ap())
            nc.sync.dma_start(out=ident_sb, in_=ident.ap())
            nc.sync.dma_start(out=brel_sb, in_=brel.ap())
            nc.sync.dma_start(out=bglob_sb, in_=bglob.ap())
            # cen48[16j+k, p] = posC[j, p]  (center replicated over k)
            src = posC_sb[0:3, :]
            nc.sync.dma_start(
                out=cen48[0:48, :],
                in_=_view(src, [[src.ap[0][0], 3], [0, 16], [1, NP]]))

            with tc.tile_pool(name="g0pool", bufs=2) as g0p, \
                 tc.tile_pool(name="gpool", bufs=3) as gp, \
                 tc.tile_pool(name="epool", bufs=2) as ep, \
                 tc.tile_pool(name="spool", bufs=2) as sp, \
                 tc.tile_pool(name="mps", bufs=2, space="PSUM") as mpsum, \
                 tc.tile_pool(name="scps", bufs=2, space="PSUM") as scpsum, \
                 tc.tile_pool(name="accps", bufs=2, space="PSUM") as apsum:

                def gather(t):
                    # raw row-gather: g0[p, s, :] = tabT[idx[s*128+p], :]
                    # split over the 4 SWDGE queues so the rings drain in
                    # parallel (each ring is serviced at ~1 DMA engine rate)
                    g0 = g0p.tile([128, PKT], BF16, tag="g0")
                    NQ, SUB = 4, PKT // 4
                    for q in range(NQ):
                        g0v = g0[:, q * SUB:(q + 1) * SUB]
                        g03 = _view(g0v, [[g0v.ap[0][0], 128],
                                          [128, SUB // 128], [1, 128]])
                        nc.gpsimd.dma_gather(
                            out_ap=g03, in_ap=tabT.ap(),
                            idxs_ap=idx_sb[:, t * TP + q * (SUB // 16):
                                           t * TP + (q + 1) * (SUB // 16)],
                            num_idxs=SUB, num_idxs_reg=SUB, elem_size=128,
                            transpose=False, single_packet=False, queue_num=q)
                    return g0

                def transpose_geom(t, g0):
                    # xbar transpose into f-layout: G[c, s*128+p] = g0[p, s*128+c]
                    g = gp.tile([128, PKT], BF16, tag="G")
                    gv = g[:, :]
                    g3 = _view(gv, [[gv.ap[0][0], 128], [128, PKT // 128],
                                    [1, 128]])
                    nc.sync.dma_start_transpose(g3, g0[:, :])

                    # geometry: dist into G row 67, center into 68:71
                    cb_src = posC_sb[0:3, t * TP:(t + 1) * TP]
                    cb_dst = g[68:71, :]
                    nc.sync.dma_start(
                        out=_view(cb_dst, [[cb_dst.ap[0][0], 3], [TP, 16],
                                           [1, TP]]),
                        in_=_view(cb_src, [[cb_src.ap[0][0], 3], [0, 16],
                                           [1, TP]]))
                    n48 = sp.tile([48, TP], BF16, tag="n48")
                    np_src = g[64:67, :]
                    nc.sync.dma_start(
                        out=n48[0:48, :],
                        in_=_view(np_src, [[np_src.ap[0][0], 3], [TP, 16],
                                           [1, TP]]))
                    nc.vector.tensor_tensor(
                        out=n48, in0=n48,
                        in1=cen48[0:48, t * TP:(t + 1) * TP], op=OP.subtract)
                    nc.vector.tensor_mul(n48, n48, n48)
                    psd = mpsum.tile([128, TP], F32, tag="rp")
                    nc.tensor.matmul(psd[0:16, :], w48_sb[0:48, :],
                                     n48[0:48, :], start=True, stop=True)
                    dsb = sp.tile([16, TP], BF16, tag="dsb")
                    nc.scalar.activation(out=dsb[0:16, :], in_=psd[0:16, :],
                                         func=AF.Sqrt)
                    d_dst = g[67:68, :]
                    nc.sync.dma_start(
                        out=_view(d_dst, [[d_dst.ap[0][0], 1], [TP, 16],
                                          [1, TP]]),
                        in_=dsb[0:16, :])
                    return g

                def epilogue(t, ps_den, ps_num):
                    rcp = sp.tile([128, TP], F32, tag="rcp")
                    nc.vector.reciprocal(rcp, ps_den)
                    agg = sp.tile([128, TP], BF16, tag="agg")
                    nc.vector.tensor_mul(agg, ps_num, rcp)
                    ps_o = mpsum.tile([128, TP], F32, tag="rp")
                    nc.tensor.matmul(ps_o, wglob_sb, agg, start=True, stop=True)
                    osb = sp.tile([128, TP], F32, tag="osb")
                    nc.scalar.activation(out=osb, in_=ps_o, func=AF.Relu,
                                         bias=bglob_sb, scale=1.0)
                    nc.sync.dma_start(out=outp.ap()[:, t * TP:(t + 1) * TP],
                                      in_=osb)

                g0_list = [gather(0), gather(1)]
                g_list = [transpose_geom(0, g0_list[0])]
                pend = None
                for t in range(NT):
                    if t + 2 < NT:
                        g0_list.append(gather(t + 2))
                    if t + 1 < NT:
                        g_list.append(transpose_geom(t + 1, g0_list[t + 1]))
                    g = g_list[t]

                    # ---- phase A: rp chunks (PE keeps w7 resident)
                    for cc in range(NCH):
                        cols = slice(cc * TP, (cc + 1) * TP)
                        ps_rp = mpsum.tile([128, TP], F32, tag="rp")
                        nc.tensor.matmul(ps_rp[64:128, :], w7_sb[64:71, :],
                                         g[64:71, cols], start=True, stop=True,
                                         tile_position=(64, 64))
                        if cc % 2 == 0:
                            nc.scalar.activation(out=g[64:128, cols],
                                                 in_=ps_rp[64:128, :],
                                                 func=AF.Relu,
                                                 bias=brel_sb[64:128, :],
                                                 scale=1.0)
                        else:
                            nc.vector.tensor_scalar(out=g[64:128, cols],
                                                    in0=ps_rp[64:128, :],
                                                    scalar1=brel_sb[64:128, :],
                                                    scalar2=0.0,
                                                    op0=OP.add, op1=OP.max)

                    # ---- phase B: attention scores / exp / f*e (f*e in-place in G)
                    eu = ep.tile([128, PKT], BF16, tag="eu")
                    for cc in range(NCH):
                        cols = slice(cc * TP, (cc + 1) * TP)
                        ps_s = scpsum.tile([128, TP], F32, tag="sc")
                        nc.tensor.matmul(ps_s, watt_sb, g[:, cols],
                                         start=True, stop=True)
                        nc.scalar.activation(out=eu[:, cols], in_=ps_s,
                                             func=AF.Exp)
                        nc.vector.tensor_mul(g[:, cols], g[:, cols],
                                             eu[:, cols])

                    # ---- phase C: accumulate num/den over k
                    ps_den = apsum.tile([128, TP], F32, tag="den")
                    ps_num = apsum.tile([128, TP], F32, tag="num")
                    for cc in range(NCH):
                        nc.tensor.matmul(ps_den, ident_sb,
                                         eu[:, cc * TP:(cc + 1) * TP],
                                         start=(cc == 0), stop=(cc == NCH - 1),
                                         skip_group_check=True)
                        nc.tensor.matmul(ps_num, ident_sb,
                                         g[:, cc * TP:(cc + 1) * TP],
                                         start=(cc == 0), stop=(cc == NCH - 1),
                                         skip_group_check=True)

                    # previous tile's softmax epilogue lands here so its
                    # reciprocal overlaps this tile's accumulation on the PE
                    if pend is not None:
                        epilogue(*pend)
                    pend = (t, ps_den, ps_num)
                epilogue(*pend)
    nc.compile()
    return nc


_NC = None


def _get_nc():
    global _NC
    if _NC is None:
        _NC = _build_kernel()
    return _NC


_PERM = (np.arange(128) + 64) % 128


def _prep_core(core, x, pos, neigh, Wc, Wn, Wr, wd, W_att, W_glob, b_rel, b_glob):
    b = core // 2
    half = core % 2
    P0 = half * NP
    nb = neigh[b][P0:P0 + NP].astype(np.int64)      # [NP, K]

    # packed gather table: row n = [x[:, n] | pos[n] | 0pad]  (bf16)
    tabT = np.zeros((N, 128), dtype=BF)
    tabT[:, 0:64] = x[b].T.astype(BF)
    tabT[:, 64:67] = pos[b].astype(BF)

    # gather idx: tile t cols (k,i) -> nb[t*512+i, k]; wrapped 16 + replicated
    A = nb.reshape(NT, TP, K)                        # [t, i, k]
    V = A.transpose(0, 2, 1).reshape(NT, PKT)        # [t, col] col=k*512+i
    W16 = V.reshape(NT, TP, 16).transpose(0, 2, 1)   # [t, j, s]: idx s*16+j
    idxg = np.tile(W16.transpose(1, 0, 2).reshape(16, NP), (8, 1)).astype(np.int16)

    posCa = pos[b][P0:P0 + NP].T.astype(BF)          # [3, NP]

    w7v = np.zeros((128, 64), dtype=BF)
    w7v[64:67] = (Wn + Wr).astype(BF)
    w7v[67:68] = wd.astype(BF)
    w7v[68:71] = (Wc - Wr).astype(BF)

    w48 = np.zeros((128, 16), dtype=BF)
    for j in range(3):
        for k in range(16):
            w48[16 * j + k, k] = 1

    brel_full = np.zeros((128, 1), np.float32)
    brel_full[64:128, 0] = b_rel

    return {
        "tabT": tabT, "idxg": idxg, "posC": posCa,
        "w7": w7v,
        "watt": W_att[np.ix_(_PERM, _PERM)].astype(BF),
        "wglob": W_glob[_PERM, :].astype(BF),
        "w48": w48,
        "ident": np.eye(128, dtype=BF),
        "brel": brel_full,
        "bglob": b_glob.reshape(128, 1).astype(np.float32),
    }


def kernel(x, pos, neigh_idx, W_rel, b_rel, W_att, W_glob, b_glob, **kw):
    x = np.ascontiguousarray(np.asarray(x, dtype=np.float32))
    pos = np.ascontiguousarray(np.asarray(pos, dtype=np.float32))
    neigh = np.asarray(neigh_idx)
    W_rel = np.asarray(W_rel, dtype=np.float32)
    W_att = np.asarray(W_att, dtype=np.float32)
    W_glob = np.asarray(W_glob, dtype=np.float32)
    b_rel = np.asarray(b_rel, dtype=np.float32)
    b_glob = np.asarray(b_glob, dtype=np.float32)
    Wc, Wn, Wr, wd = W_rel[0:3], W_rel[3:6], W_rel[6:9], W_rel[9:10]

    nc = _get_nc()
    in_maps = [
        _prep_core(core, x, pos, neigh, Wc, Wn, Wr, wd, W_att, W_glob, b_rel, b_glob)
        for core in range(8)
    ]
    res = run_bass_kernel_spmd(nc, in_maps, core_ids=list(range(8)))
    out = np.zeros((B, C_OUT, N), np.float32)
    for core in range(8):
        b = core // 2
        P0 = (core % 2) * NP
        out[b, :, P0:P0 + NP] = res.results[core]["outp"]
    return out
